# revision 39
# baseline (speedup 1.0000x reference)
"""Trainium2 Bass kernel for nn_LogicGatedSNN.

reference semantics (single step SNN update):
    w        = (states > 50)                      # [O, I] ternary weights
    current  = w @ spike_input                    # [O] GEMV
    v_new    = v_mem * 0.8 + current + noise
    spikes   = (v_new >= v_th)
    elig_new = clip(elig * 0.95 + outer(spikes, spike_input), 0, 5)
    v_th_new = clip(v_th + (spikes - 0.05) * 0.1, 0.5, 10)
    v_mem_new = v_new * (1 - spikes) * 0.2
    returns (spikes, v_mem_new, v_th_new, elig_new)

Sharding: rows (out_features) split across 8 cores; spike_input replicated.
No cross-core communication.

Device kernel per core (o_shard = 1024 rows = 8 tiles of 128 partitions):
    scalar_tensor_tensor: G = (states is_gt 50) * spike_bcast,
                          accum_out = row-sum -> current   (ONE DVE pass)
    tiny [128,1] DVE ops for v_new / spikes / v_th_new / v_mem_new
    outer(spikes, spike) on the Scalar engine:
                          activation(Copy, in=spike_bcast, scale=spikes_col)

Fast mode (dispatched only after the host verifies elig is exactly all-zero
and spike_input is exactly binary): skips the elig read; elig_new values are
then exactly {0,1}, so it is stored as fp8-e4 (exact) and spike_input is
carried as fp8 (exact for binary) and replicated across partitions on the
Tensor engine — pure bandwidth savings, bit-identical results after the host
upcasts to float32. Per-core HBM traffic is 32 MB states in + 8 MB fp8
elig_new out, streamed at ~400 GB/s; typical HW time ~121-130 us.

General mode handles arbitrary inputs in full float32.
"""

import contextlib

import numpy as np
import ml_dtypes

import concourse.bacc as bacc
import concourse.mybir as mybir
from concourse import tile
from concourse.bass_utils import run_bass_kernel_spmd

N_CORES = 8
OUT_DIM = 8192
IN_DIM = 8192
P = 128
THRESHOLD = 50.0

F32 = mybir.dt.float32
BF16 = mybir.dt.bfloat16
FP8 = mybir.dt.float8e4
Op = mybir.AluOpType
Act = mybir.ActivationFunctionType


def _small_vec_ops(nc, tp, vec_t, sv_t, current, t, n_tiles):
    """[128,1] ops for one row-tile: v_new, spikes, v_th_new, v_mem_new.

    Arithmetic matches the reference's op order exactly (bit-identical fp32).
    Returns the spikes column AP.
    """
    vm = vec_t[:, t:t + 1]
    vt = vec_t[:, n_tiles + t:n_tiles + t + 1]
    nz = vec_t[:, 2 * n_tiles + t:2 * n_tiles + t + 1]

    # v_new = (v_mem * 0.8 + current) + noise
    a = tp.tile([P, 1], F32, tag="a")
    nc.vector.scalar_tensor_tensor(out=a[:], in0=vm, scalar=0.8,
                                   in1=current[:], op0=Op.mult, op1=Op.add)
    v_new = tp.tile([P, 1], F32, tag="v_new")
    nc.vector.tensor_tensor(out=v_new[:], in0=a[:], in1=nz, op=Op.add)

    # spikes = v_new >= v_th
    spk = sv_t[:, t:t + 1]
    nc.vector.tensor_tensor(out=spk, in0=v_new[:], in1=vt, op=Op.is_ge)

    # v_th_new = clip(v_th + (spikes - 0.05) * 0.1, 0.5, 10)
    d = tp.tile([P, 1], F32, tag="d")
    nc.vector.tensor_scalar(out=d[:], in0=spk, scalar1=-0.05,
                            scalar2=0.1, op0=Op.add, op1=Op.mult)
    e = tp.tile([P, 1], F32, tag="e")
    nc.vector.tensor_tensor(out=e[:], in0=vt, in1=d[:], op=Op.add)
    nc.vector.tensor_scalar(
        out=sv_t[:, 2 * n_tiles + t:2 * n_tiles + t + 1], in0=e[:],
        scalar1=0.5, scalar2=10.0, op0=Op.max, op1=Op.min)

    # v_mem_new = (v_new * (1 - spikes)) * 0.2
    ns = tp.tile([P, 1], F32, tag="ns")
    nc.vector.tensor_scalar(out=ns[:], in0=spk, scalar1=-1.0,
                            scalar2=1.0, op0=Op.mult, op1=Op.add)
    f = tp.tile([P, 1], F32, tag="f")
    nc.vector.tensor_tensor(out=f[:], in0=v_new[:], in1=ns[:], op=Op.mult)
    nc.vector.tensor_scalar(
        out=sv_t[:, n_tiles + t:n_tiles + t + 1], in0=f[:],
        scalar1=0.2, scalar2=None, op0=Op.mult)
    return spk


def build_fast_program(o_shard=OUT_DIM // N_CORES, in_dim=IN_DIM):
    """elig==0, binary spike_input: skip elig read, fp8 output, fp8 spike.

    Layout: states rows -> 8 tiles of 128 partitions; one fused DVE
    scalar_tensor_tensor per tile computes (states > 50) * spike AND its
    row-sum. The 8 KB spike row is replicated across partitions on the idle
    Tensor engine (rank-1 ones matmul -> PSUM -> ACT copy), saving the 1 MB
    broadcast DMA. The first row-tile is column-chunked (ascending widths)
    so compute starts ~1.3 MB into the load stream; the last row-tile is
    chunked descending and its outer product is split DVE/ACT with stores
    routed to the then-idle sync ring to shorten the post-stream tail.
    elig_new values are exactly {0,1}: stored fp8, host upcasts.
    """
    n_tiles = o_shard // P
    nv = 3 * n_tiles
    asc = [in_dim // 32, in_dim // 8, in_dim * 11 // 32, in_dim // 2]
    NCH = len(asc)
    asc_off = [0]
    for w in asc:
        asc_off.append(asc_off[-1] + w)
    desc = asc[::-1]
    desc_off = [0]
    for w in desc:
        desc_off.append(desc_off[-1] + w)

    nc = bacc.Bacc("TRN2", target_bir_lowering=False, debug=False)
    states = nc.dram_tensor("states", [o_shard, in_dim], F32, kind="ExternalInput")
    spike_b = nc.dram_tensor("spike_b", [1, in_dim], FP8, kind="ExternalInput")
    vecs = nc.dram_tensor("vecs", [P, nv], F32, kind="ExternalInput")
    elig_new = nc.dram_tensor("elig_new", [o_shard, in_dim], FP8, kind="ExternalOutput")
    svec = nc.dram_tensor("svec", [P, nv], F32, kind="ExternalOutput")

    with tile.TileContext(nc) as tc:
        with contextlib.ExitStack() as ctx:
            constp = ctx.enter_context(tc.tile_pool(name="constp", bufs=1))
            sp = ctx.enter_context(tc.tile_pool(name="sp", bufs=3))
            scp = ctx.enter_context(tc.tile_pool(name="scp", bufs=2))
            gp = ctx.enter_context(tc.tile_pool(name="gp", bufs=1))
            outp = ctx.enter_context(tc.tile_pool(name="outp", bufs=2))
            tp = ctx.enter_context(tc.tile_pool(name="tp", bufs=3))

            spike_t = constp.tile([P, in_dim], FP8, tag="spike_t")
            vec_t = constp.tile([P, nv], F32, tag="vec_t")
            sv_t = constp.tile([P, nv], F32, tag="sv_t")

            sp_row = constp.tile([1, in_dim], FP8, tag="sp_row")
            nc.sync.dma_start(out=sp_row[:], in_=spike_b[:])
            ones = constp.tile([1, P], FP8, tag="ones")
            nc.vector.memset(ones[:], 1.0)
            psp = ctx.enter_context(tc.tile_pool(name="psp", bufs=2, space="PSUM"))
            PW = min(2048, in_dim)
            NMM = 512
            for j in range(in_dim // PW):
                pt = psp.tile([P, PW], F32, tag="pt")
                for k in range(PW // NMM):
                    cols = slice(j * PW + k * NMM, j * PW + (k + 1) * NMM)
                    nc.tensor.matmul(pt[:, k * NMM:(k + 1) * NMM],
                                     ones[:], sp_row[:, cols],
                                     start=True, stop=True)
                nc.scalar.activation(out=spike_t[:, j * PW:(j + 1) * PW],
                                     in_=pt[:], func=Act.Copy)

            first_S = [None] * NCH
            first_S[0] = scp.tile([P, asc[0]], F32, tag="Sc", name="S0c0")
            nc.sync.dma_start(out=first_S[0][:], in_=states[0:P, 0:asc[0]])
            nc.sync.dma_start(out=vec_t[:], in_=vecs[:])
            for c in range(1, NCH):
                cols = slice(asc_off[c], asc_off[c + 1])
                first_S[c] = scp.tile([P, asc[c]], F32, tag="Sc", name=f"S0c{c}")
                nc.sync.dma_start(out=first_S[c][:], in_=states[0:P, cols])

            def chunked_current(S_chunks, spike_ap, offs):
                curs = []
                for c, Sc in enumerate(S_chunks):
                    cols = slice(offs[c], offs[c + 1])
                    Gc = gp.tile([P, offs[c + 1] - offs[c]], FP8, tag="Gc",
                                 bufs=2)
                    curc = tp.tile([P, 1], F32, tag="curc", bufs=2 * NCH)
                    nc.vector.scalar_tensor_tensor(
                        out=Gc[:], in0=Sc[:], scalar=THRESHOLD,
                        in1=spike_ap[:, cols],
                        op0=Op.is_gt, op1=Op.mult, accum_out=curc[:])
                    curs.append(curc)
                current = curs[0]
                for other in curs[1:]:
                    acc = tp.tile([P, 1], F32, tag="curacc", bufs=NCH)
                    nc.vector.tensor_tensor(out=acc[:], in0=current[:],
                                            in1=other[:], op=Op.add)
                    current = acc
                return current

            for t in range(n_tiles):
                rows = slice(t * P, (t + 1) * P)
                last = t == n_tiles - 1

                if t == 0:
                    current = chunked_current(first_S, spike_t, asc_off)
                elif last and n_tiles > 1:
                    S_chunks = []
                    for c in range(NCH):
                        cols = slice(desc_off[c], desc_off[c + 1])
                        Sc = scp.tile([P, desc[c]], F32, tag="Sc", name=f"SLc{c}")
                        nc.sync.dma_start(out=Sc[:], in_=states[rows, cols])
                        S_chunks.append(Sc)
                    current = chunked_current(S_chunks, spike_t, desc_off)
                else:
                    S = sp.tile([P, in_dim], F32, tag="S")
                    nc.sync.dma_start(out=S[:], in_=states[rows, :])
                    G = gp.tile([P, in_dim], FP8, tag="G")
                    cur = tp.tile([P, 1], F32, tag="cur")
                    nc.vector.scalar_tensor_tensor(
                        out=G[:], in0=S[:], scalar=THRESHOLD, in1=spike_t[:],
                        op0=Op.is_gt, op1=Op.mult, accum_out=cur[:])
                    current = cur

                spk = _small_vec_ops(nc, tp, vec_t, sv_t, current, t, n_tiles)

                if last:
                    nc.sync.dma_start(out=svec[:], in_=sv_t[:])
                    O = outp.tile([P, in_dim], FP8, tag="O")
                    for c in range(NCH):
                        cols = slice(desc_off[c], desc_off[c + 1])
                        if c < 2:
                            nc.vector.tensor_scalar(out=O[:, cols],
                                                    in0=spike_t[:, cols],
                                                    scalar1=spk, scalar2=None,
                                                    op0=Op.mult)
                        else:
                            nc.scalar.activation(out=O[:, cols],
                                                 in_=spike_t[:, cols],
                                                 func=Act.Copy, scale=spk)
                        nc.sync.dma_start(out=elig_new[rows, cols],
                                          in_=O[:, cols])
                else:
                    O = outp.tile([P, in_dim], FP8, tag="O")
                    nc.scalar.activation(out=O[:], in_=spike_t[:], func=Act.Copy,
                                         scale=spk)
                    nc.scalar.dma_start(out=elig_new[rows, :], in_=O[:])

    nc.compile()
    nc._spike_rows = 1
    nc._w0 = 0
    return nc


def build_general_program(o_shard=OUT_DIM // N_CORES, in_dim=IN_DIM, chunk=4096):
    """Arbitrary inputs, full float32."""
    n_tiles = o_shard // P
    n_chunks = in_dim // chunk
    nv = 3 * n_tiles

    nc = bacc.Bacc("TRN2", target_bir_lowering=False, debug=False)
    states = nc.dram_tensor("states", [o_shard, in_dim], F32, kind="ExternalInput")
    spike_b = nc.dram_tensor("spike_b", [P, in_dim], F32, kind="ExternalInput")
    vecs = nc.dram_tensor("vecs", [P, nv], F32, kind="ExternalInput")
    elig = nc.dram_tensor("elig", [o_shard, in_dim], F32, kind="ExternalInput")
    elig_new = nc.dram_tensor("elig_new", [o_shard, in_dim], F32, kind="ExternalOutput")
    svec = nc.dram_tensor("svec", [P, nv], F32, kind="ExternalOutput")

    with tile.TileContext(nc) as tc:
        with contextlib.ExitStack() as ctx:
            constp = ctx.enter_context(tc.tile_pool(name="constp", bufs=1))
            sp = ctx.enter_context(tc.tile_pool(name="sp", bufs=2))
            gp = ctx.enter_context(tc.tile_pool(name="gp", bufs=1))
            outp = ctx.enter_context(tc.tile_pool(name="outp", bufs=2))
            ep = ctx.enter_context(tc.tile_pool(name="ep", bufs=2))
            e2p = ctx.enter_context(tc.tile_pool(name="e2p", bufs=2))
            tp = ctx.enter_context(tc.tile_pool(name="tp", bufs=3))

            spike_t = constp.tile([P, in_dim], F32, tag="spike_t")
            nc.sync.dma_start(out=spike_t[:], in_=spike_b[:])
            vec_t = constp.tile([P, nv], F32, tag="vec_t")
            nc.sync.dma_start(out=vec_t[:], in_=vecs[:])
            sv_t = constp.tile([P, nv], F32, tag="sv_t")

            for t in range(n_tiles):
                rows = slice(t * P, (t + 1) * P)
                curs = []
                for c in range(n_chunks):
                    cols = slice(c * chunk, (c + 1) * chunk)
                    S = sp.tile([P, chunk], F32, tag="S")
                    nc.sync.dma_start(out=S[:], in_=states[rows, cols])
                    G = gp.tile([P, chunk], F32, tag="G")
                    cur = tp.tile([P, 1], F32, tag="cur", bufs=2 * n_chunks)
                    nc.vector.scalar_tensor_tensor(
                        out=G[:], in0=S[:], scalar=THRESHOLD,
                        in1=spike_t[:, cols],
                        op0=Op.is_gt, op1=Op.mult, accum_out=cur[:])
                    curs.append(cur)
                current = curs[0]
                for other in curs[1:]:
                    acc = tp.tile([P, 1], F32, tag="curacc", bufs=2)
                    nc.vector.tensor_tensor(out=acc[:], in0=current[:],
                                            in1=other[:], op=Op.add)
                    current = acc

                spk = _small_vec_ops(nc, tp, vec_t, sv_t, current, t, n_tiles)

                for c in range(n_chunks):
                    cols = slice(c * chunk, (c + 1) * chunk)
                    O = outp.tile([P, chunk], F32, tag="O")
                    nc.scalar.activation(out=O[:], in_=spike_t[:, cols],
                                         func=Act.Copy, scale=spk)
                    E = ep.tile([P, chunk], F32, tag="E")
                    nc.sync.dma_start(out=E[:], in_=elig[rows, cols])
                    E2 = e2p.tile([P, chunk], F32, tag="E2")
                    nc.vector.scalar_tensor_tensor(
                        out=E2[:], in0=E[:], scalar=0.95, in1=O[:],
                        op0=Op.mult, op1=Op.add)
                    nc.vector.tensor_scalar(out=E2[:], in0=E2[:],
                                            scalar1=0.0, scalar2=5.0,
                                            op0=Op.max, op1=Op.min)
                    nc.scalar.dma_start(out=elig_new[rows, cols], in_=E2[:])

            nc.scalar.dma_start(out=svec[:], in_=sv_t[:])

    nc.compile()
    return nc


_PROGRAM_CACHE = {}


def _get_program(fast: bool):
    if fast not in _PROGRAM_CACHE:
        _PROGRAM_CACHE[fast] = (build_fast_program() if fast
                                else build_general_program())
    return _PROGRAM_CACHE[fast]


def _pack_vec(v, n_tiles):
    # [o_shard] -> [128, n_tiles] with column t, partition p = v[t*128+p]
    return np.ascontiguousarray(v.reshape(n_tiles, P).T)


def _unpack_vec(m, n_tiles):
    # inverse of _pack_vec
    return np.ascontiguousarray(m.T).reshape(n_tiles * P)


def run(inputs, trace=False):
    spike_input = np.ascontiguousarray(np.asarray(inputs["spike_input"], dtype=np.float32))
    states = np.ascontiguousarray(np.asarray(inputs["states"], dtype=np.float32))
    v_mem = np.asarray(inputs["v_mem"], dtype=np.float32)
    v_th = np.asarray(inputs["v_th"], dtype=np.float32)
    elig = np.asarray(inputs["elig"], dtype=np.float32)
    noise = np.asarray(inputs["noise"], dtype=np.float32)

    o_shard = OUT_DIM // N_CORES
    n_tiles = o_shard // P

    spike_binary = bool(((spike_input == 0.0) | (spike_input == 1.0)).all())
    fast = (not elig.any()) and spike_binary
    nc = _get_program(fast)

    if fast:
        rows = getattr(nc, "_spike_rows", 1)
        spike_f8 = spike_input.astype(ml_dtypes.float8_e4m3)
        spike_b = np.ascontiguousarray(np.broadcast_to(spike_f8, (rows, IN_DIM)))
    else:
        spike_b = np.ascontiguousarray(np.broadcast_to(spike_input, (P, IN_DIM)))

    in_maps = []
    for c in range(N_CORES):
        rows = slice(c * o_shard, (c + 1) * o_shard)
        vecs = np.concatenate(
            [_pack_vec(v_mem[rows], n_tiles),
             _pack_vec(v_th[rows], n_tiles),
             _pack_vec(noise[rows], n_tiles)], axis=1).astype(np.float32)
        m = {
            "states": np.ascontiguousarray(states[rows]),
            "spike_b": spike_b,
            "vecs": np.ascontiguousarray(vecs),
        }
        if not fast:
            m["elig"] = np.ascontiguousarray(elig[rows])
        in_maps.append(m)

    res = run_bass_kernel_spmd(nc, in_maps, list(range(N_CORES)), trace=trace)

    spikes = np.empty(OUT_DIM, dtype=np.float32)
    v_mem_new = np.empty(OUT_DIM, dtype=np.float32)
    v_th_new = np.empty(OUT_DIM, dtype=np.float32)
    elig_new = np.empty((OUT_DIM, IN_DIM), dtype=np.float32)
    for c in range(N_CORES):
        rows = slice(c * o_shard, (c + 1) * o_shard)
        out = res.results[c]
        sv = out["svec"]
        spikes[rows] = _unpack_vec(sv[:, 0:n_tiles], n_tiles)
        v_mem_new[rows] = _unpack_vec(sv[:, n_tiles:2 * n_tiles], n_tiles)
        v_th_new[rows] = _unpack_vec(sv[:, 2 * n_tiles:3 * n_tiles], n_tiles)
        elig_new[rows] = np.asarray(out["elig_new"]).astype(np.float32)

    return (spikes, v_mem_new, v_th_new, elig_new), res


def kernel(**inputs):
    outputs, _ = run(inputs, trace=False)
    return outputs


# revision 43
# speedup vs baseline: 1.2717x; 1.2717x over previous
"""Trainium2 Bass kernel for nn_LogicGatedSNN.

reference semantics (single step SNN update):
    w        = (states > 50)                      # [O, I] ternary weights
    current  = w @ spike_input                    # [O] GEMV
    v_new    = v_mem * 0.8 + current + noise
    spikes   = (v_new >= v_th)
    elig_new = clip(elig * 0.95 + outer(spikes, spike_input), 0, 5)
    v_th_new = clip(v_th + (spikes - 0.05) * 0.1, 0.5, 10)
    v_mem_new = v_new * (1 - spikes) * 0.2
    returns (spikes, v_mem_new, v_th_new, elig_new)

Sharding: rows (out_features) split across 8 cores; spike_input replicated.
No cross-core communication.

Device kernel per core (o_shard = 1024 rows = 8 tiles of 128 partitions):
    scalar_tensor_tensor: G = (states is_gt 50) * spike_bcast,
                          accum_out = row-sum -> current   (ONE DVE pass)
    tiny [128,1] DVE ops for v_new / spikes / v_th_new / v_mem_new
    outer(spikes, spike) on the Scalar engine:
                          activation(Copy, in=spike_bcast, scale=spikes_col)

Fast mode (dispatched only after the host verifies elig is exactly all-zero
and spike_input is exactly binary): skips the elig read; elig_new values are
then exactly {0,1}, so it is stored as fp8-e4 (exact) and spike_input is
carried as fp8 (exact for binary) and replicated across partitions on the
Tensor engine — pure bandwidth savings, bit-identical results after the host
upcasts to float32. Per-core HBM traffic is 32 MB states in + 8 MB fp8
elig_new out, streamed at ~400 GB/s; typical HW time ~121-130 us.

General mode handles arbitrary inputs in full float32.
"""

import contextlib

import numpy as np
import ml_dtypes

import concourse.bacc as bacc
import concourse.mybir as mybir
from concourse import tile
from concourse.bass_utils import run_bass_kernel_spmd

N_CORES = 8
OUT_DIM = 8192
IN_DIM = 8192
P = 128
THRESHOLD = 50.0

F32 = mybir.dt.float32
BF16 = mybir.dt.bfloat16
FP8 = mybir.dt.float8e4
Op = mybir.AluOpType
Act = mybir.ActivationFunctionType


def _small_vec_ops(nc, tp, vec_t, sv_t, current, t, n_tiles):
    """[128,1] ops for one row-tile: v_new, spikes, v_th_new, v_mem_new.

    Arithmetic matches the reference's op order exactly (bit-identical fp32).
    Returns the spikes column AP.
    """
    vm = vec_t[:, t:t + 1]
    vt = vec_t[:, n_tiles + t:n_tiles + t + 1]
    nz = vec_t[:, 2 * n_tiles + t:2 * n_tiles + t + 1]

    # v_new = (v_mem * 0.8 + current) + noise
    a = tp.tile([P, 1], F32, tag="a")
    nc.vector.scalar_tensor_tensor(out=a[:], in0=vm, scalar=0.8,
                                   in1=current[:], op0=Op.mult, op1=Op.add)
    v_new = tp.tile([P, 1], F32, tag="v_new")
    nc.vector.tensor_tensor(out=v_new[:], in0=a[:], in1=nz, op=Op.add)

    # spikes = v_new >= v_th
    spk = sv_t[:, t:t + 1]
    nc.vector.tensor_tensor(out=spk, in0=v_new[:], in1=vt, op=Op.is_ge)

    # v_th_new = clip(v_th + (spikes - 0.05) * 0.1, 0.5, 10)
    d = tp.tile([P, 1], F32, tag="d")
    nc.vector.tensor_scalar(out=d[:], in0=spk, scalar1=-0.05,
                            scalar2=0.1, op0=Op.add, op1=Op.mult)
    e = tp.tile([P, 1], F32, tag="e")
    nc.vector.tensor_tensor(out=e[:], in0=vt, in1=d[:], op=Op.add)
    nc.vector.tensor_scalar(
        out=sv_t[:, 2 * n_tiles + t:2 * n_tiles + t + 1], in0=e[:],
        scalar1=0.5, scalar2=10.0, op0=Op.max, op1=Op.min)

    # v_mem_new = (v_new * (1 - spikes)) * 0.2
    ns = tp.tile([P, 1], F32, tag="ns")
    nc.vector.tensor_scalar(out=ns[:], in0=spk, scalar1=-1.0,
                            scalar2=1.0, op0=Op.mult, op1=Op.add)
    f = tp.tile([P, 1], F32, tag="f")
    nc.vector.tensor_tensor(out=f[:], in0=v_new[:], in1=ns[:], op=Op.mult)
    nc.vector.tensor_scalar(
        out=sv_t[:, n_tiles + t:n_tiles + t + 1], in0=f[:],
        scalar1=0.2, scalar2=None, op0=Op.mult)
    return spk


def build_fast_program(o_shard=OUT_DIM // N_CORES, in_dim=IN_DIM):
    """elig==0, binary spike_input: skip elig read, fp8 output, fp8 spike.

    Layout: states rows -> 8 tiles of 128 partitions; one fused DVE
    scalar_tensor_tensor per tile computes (states > 50) * spike AND its
    row-sum. The 8 KB spike row is replicated across partitions on the idle
    Tensor engine (rank-1 ones matmul -> PSUM -> ACT copy), saving the 1 MB
    broadcast DMA. The first row-tile is column-chunked (ascending widths)
    so compute starts ~1.3 MB into the load stream; the last row-tile is
    chunked descending and its outer product is split DVE/ACT with stores
    routed to the then-idle sync ring to shorten the post-stream tail.
    elig_new values are exactly {0,1}: stored fp8, host upcasts.
    """
    n_tiles = o_shard // P
    nv = 3 * n_tiles
    asc = [in_dim // 32, in_dim // 8, in_dim * 11 // 32, in_dim // 2]
    NCH = len(asc)
    asc_off = [0]
    for w in asc:
        asc_off.append(asc_off[-1] + w)
    desc = asc[::-1]
    desc_off = [0]
    for w in desc:
        desc_off.append(desc_off[-1] + w)

    nc = bacc.Bacc("TRN2", target_bir_lowering=False, debug=False)
    states = nc.dram_tensor("states", [o_shard, in_dim], F32, kind="ExternalInput")
    spike_b = nc.dram_tensor("spike_b", [1, in_dim], FP8, kind="ExternalInput")
    vecs = nc.dram_tensor("vecs", [P, nv], F32, kind="ExternalInput")
    elig_new = nc.dram_tensor("elig_new", [o_shard, in_dim], FP8, kind="ExternalOutput")
    svec = nc.dram_tensor("svec", [P, nv], F32, kind="ExternalOutput")

    with tile.TileContext(nc) as tc:
        with contextlib.ExitStack() as ctx:
            constp = ctx.enter_context(tc.tile_pool(name="constp", bufs=1))
            sp = ctx.enter_context(tc.tile_pool(name="sp", bufs=3))
            scp = ctx.enter_context(tc.tile_pool(name="scp", bufs=2))
            gp = ctx.enter_context(tc.tile_pool(name="gp", bufs=1))
            outp = ctx.enter_context(tc.tile_pool(name="outp", bufs=2))
            tp = ctx.enter_context(tc.tile_pool(name="tp", bufs=3))

            spike_t = constp.tile([P, in_dim], FP8, tag="spike_t")
            vec_t = constp.tile([P, nv], F32, tag="vec_t")
            sv_t = constp.tile([P, nv], F32, tag="sv_t")

            sp_row = constp.tile([1, in_dim], FP8, tag="sp_row")
            nc.sync.dma_start(out=sp_row[:], in_=spike_b[:])
            ones = constp.tile([1, P], FP8, tag="ones")
            nc.vector.memset(ones[:], 1.0)
            psp = ctx.enter_context(tc.tile_pool(name="psp", bufs=2, space="PSUM"))
            PW = min(2048, in_dim)
            NMM = 512
            for j in range(in_dim // PW):
                pt = psp.tile([P, PW], F32, tag="pt")
                for k in range(PW // NMM):
                    cols = slice(j * PW + k * NMM, j * PW + (k + 1) * NMM)
                    nc.tensor.matmul(pt[:, k * NMM:(k + 1) * NMM],
                                     ones[:], sp_row[:, cols],
                                     start=True, stop=True)
                nc.scalar.activation(out=spike_t[:, j * PW:(j + 1) * PW],
                                     in_=pt[:], func=Act.Copy)

            first_S = [None] * NCH
            first_S[0] = scp.tile([P, asc[0]], F32, tag="Sc", name="S0c0")
            nc.sync.dma_start(out=first_S[0][:], in_=states[0:P, 0:asc[0]])
            nc.sync.dma_start(out=vec_t[:], in_=vecs[:])
            for c in range(1, NCH):
                cols = slice(asc_off[c], asc_off[c + 1])
                first_S[c] = scp.tile([P, asc[c]], F32, tag="Sc", name=f"S0c{c}")
                nc.sync.dma_start(out=first_S[c][:], in_=states[0:P, cols])

            def chunked_current(S_chunks, spike_ap, offs):
                curs = []
                for c, Sc in enumerate(S_chunks):
                    cols = slice(offs[c], offs[c + 1])
                    Gc = gp.tile([P, offs[c + 1] - offs[c]], FP8, tag="Gc",
                                 bufs=2)
                    curc = tp.tile([P, 1], F32, tag="curc", bufs=2 * NCH)
                    nc.vector.scalar_tensor_tensor(
                        out=Gc[:], in0=Sc[:], scalar=THRESHOLD,
                        in1=spike_ap[:, cols],
                        op0=Op.is_gt, op1=Op.mult, accum_out=curc[:])
                    curs.append(curc)
                current = curs[0]
                for other in curs[1:]:
                    acc = tp.tile([P, 1], F32, tag="curacc", bufs=NCH)
                    nc.vector.tensor_tensor(out=acc[:], in0=current[:],
                                            in1=other[:], op=Op.add)
                    current = acc
                return current

            for t in range(n_tiles):
                rows = slice(t * P, (t + 1) * P)
                last = t == n_tiles - 1

                if t == 0:
                    current = chunked_current(first_S, spike_t, asc_off)
                elif last and n_tiles > 1:
                    S_chunks = []
                    for c in range(NCH):
                        cols = slice(desc_off[c], desc_off[c + 1])
                        Sc = scp.tile([P, desc[c]], F32, tag="Sc", name=f"SLc{c}")
                        nc.sync.dma_start(out=Sc[:], in_=states[rows, cols])
                        S_chunks.append(Sc)
                    current = chunked_current(S_chunks, spike_t, desc_off)
                else:
                    S = sp.tile([P, in_dim], F32, tag="S")
                    nc.sync.dma_start(out=S[:], in_=states[rows, :])
                    G = gp.tile([P, in_dim], FP8, tag="G")
                    cur = tp.tile([P, 1], F32, tag="cur")
                    nc.vector.scalar_tensor_tensor(
                        out=G[:], in0=S[:], scalar=THRESHOLD, in1=spike_t[:],
                        op0=Op.is_gt, op1=Op.mult, accum_out=cur[:])
                    current = cur

                spk = _small_vec_ops(nc, tp, vec_t, sv_t, current, t, n_tiles)

                if last:
                    nc.sync.dma_start(out=svec[:], in_=sv_t[:])
                    O = outp.tile([P, in_dim], FP8, tag="O")
                    for c in range(NCH):
                        cols = slice(desc_off[c], desc_off[c + 1])
                        if c < 2:
                            nc.vector.tensor_scalar(out=O[:, cols],
                                                    in0=spike_t[:, cols],
                                                    scalar1=spk, scalar2=None,
                                                    op0=Op.mult)
                        else:
                            nc.scalar.activation(out=O[:, cols],
                                                 in_=spike_t[:, cols],
                                                 func=Act.Copy, scale=spk)
                        nc.sync.dma_start(out=elig_new[rows, cols],
                                          in_=O[:, cols])
                else:
                    O = outp.tile([P, in_dim], FP8, tag="O")
                    nc.scalar.activation(out=O[:], in_=spike_t[:], func=Act.Copy,
                                         scale=spk)
                    nc.scalar.dma_start(out=elig_new[rows, :], in_=O[:])

    nc.compile()
    nc._spike_rows = 1
    nc._w0 = 0
    return nc



def build_fast_program_pack(o_shard=OUT_DIM // N_CORES, in_dim=IN_DIM):
    """Like build_fast_program, but elig_new leaves the device bit-packed.

    packbits(outer(spikes, spike)) is itself rank-1: pb[j] * spike[i], where
    pb[j] = sum_k 2^k * spikes[8j+k] — 16 bytes per row-tile, computed by one
    PE matmul (pack-weight matrix @ spikes column, landing on partitions
    16t..16t+15). The ACT outer then emits the packed [16, in_dim] uint8
    block at the same engine cost, cutting the store from 1 MB to 128 KB per
    row-tile. Values are exact integers 0..255; the host unpacks bits.
    """
    n_tiles = o_shard // P
    nv = 3 * n_tiles
    asc = [in_dim // 32, in_dim // 8, in_dim * 11 // 32, in_dim // 2]
    NCH = len(asc)
    asc_off = [0]
    for w in asc:
        asc_off.append(asc_off[-1] + w)
    desc = asc[::-1]
    desc_off = [0]
    for w in desc:
        desc_off.append(desc_off[-1] + w)

    nc = bacc.Bacc("TRN2", target_bir_lowering=False, debug=False)
    states = nc.dram_tensor("states", [o_shard, in_dim], F32, kind="ExternalInput")
    spike_b = nc.dram_tensor("spike_b", [1, in_dim], FP8, kind="ExternalInput")
    vecs = nc.dram_tensor("vecs", [P, nv], F32, kind="ExternalInput")
    packm = nc.dram_tensor("packm", [P, 16], F32, kind="ExternalInput")
    elig_pack = nc.dram_tensor("elig_pack", [n_tiles * 16, in_dim],
                               mybir.dt.uint8, kind="ExternalOutput")
    svec = nc.dram_tensor("svec", [P, nv], F32, kind="ExternalOutput")

    with tile.TileContext(nc) as tc:
        with contextlib.ExitStack() as ctx:
            constp = ctx.enter_context(tc.tile_pool(name="constp", bufs=1))
            sp = ctx.enter_context(tc.tile_pool(name="sp", bufs=3))
            scp = ctx.enter_context(tc.tile_pool(name="scp", bufs=2))
            gp = ctx.enter_context(tc.tile_pool(name="gp", bufs=1))
            pkp = ctx.enter_context(tc.tile_pool(name="pkp", bufs=1))
            tp = ctx.enter_context(tc.tile_pool(name="tp", bufs=3))

            spike_t = constp.tile([P, in_dim], FP8, tag="spike_t")
            vec_t = constp.tile([P, nv], F32, tag="vec_t")
            sv_t = constp.tile([P, nv], F32, tag="sv_t")
            packm_t = constp.tile([P, 16], F32, tag="packm_t")

            sp_row = constp.tile([1, in_dim], FP8, tag="sp_row")
            nc.sync.dma_start(out=sp_row[:], in_=spike_b[:])
            nc.sync.dma_start(out=packm_t[:], in_=packm[:])
            ones = constp.tile([1, P], FP8, tag="ones")
            nc.vector.memset(ones[:], 1.0)
            psp = ctx.enter_context(tc.tile_pool(name="psp", bufs=1, space="PSUM"))
            PW = min(2048, in_dim)
            NMM = 512
            for j in range(in_dim // PW):
                pt = psp.tile([P, PW], F32, tag="pt")
                for k in range(PW // NMM):
                    cols = slice(j * PW + k * NMM, j * PW + (k + 1) * NMM)
                    nc.tensor.matmul(pt[:, k * NMM:(k + 1) * NMM],
                                     ones[:], sp_row[:, cols],
                                     start=True, stop=True)
                nc.scalar.activation(out=spike_t[:, j * PW:(j + 1) * PW],
                                     in_=pt[:], func=Act.Copy)
            pb_ps = psp.tile([16, 1], F32, tag="pb_ps", bufs=2)

            first_S = [None] * NCH
            first_S[0] = scp.tile([P, asc[0]], F32, tag="Sc", name="S0c0")
            nc.sync.dma_start(out=first_S[0][:], in_=states[0:P, 0:asc[0]])
            nc.sync.dma_start(out=vec_t[:], in_=vecs[:])
            for c in range(1, NCH):
                cols = slice(asc_off[c], asc_off[c + 1])
                first_S[c] = scp.tile([P, asc[c]], F32, tag="Sc", name=f"S0c{c}")
                nc.sync.dma_start(out=first_S[c][:], in_=states[0:P, cols])

            def chunked_current(S_chunks, spike_ap, offs):
                curs = []
                for c, Sc in enumerate(S_chunks):
                    cols = slice(offs[c], offs[c + 1])
                    Gc = gp.tile([P, offs[c + 1] - offs[c]], FP8, tag="Gc",
                                 bufs=2)
                    curc = tp.tile([P, 1], F32, tag="curc", bufs=2 * NCH)
                    nc.vector.scalar_tensor_tensor(
                        out=Gc[:], in0=Sc[:], scalar=THRESHOLD,
                        in1=spike_ap[:, cols],
                        op0=Op.is_gt, op1=Op.mult, accum_out=curc[:])
                    curs.append(curc)
                current = curs[0]
                for other in curs[1:]:
                    acc = tp.tile([P, 1], F32, tag="curacc", bufs=NCH)
                    nc.vector.tensor_tensor(out=acc[:], in0=current[:],
                                            in1=other[:], op=Op.add)
                    current = acc
                return current

            for t in range(n_tiles):
                rows = slice(t * P, (t + 1) * P)
                last = t == n_tiles - 1
                prows = slice(t * 16, (t + 1) * 16)

                if t == 0:
                    current = chunked_current(first_S, spike_t, asc_off)
                elif last and n_tiles > 1:
                    S_chunks = []
                    for c in range(NCH):
                        cols = slice(desc_off[c], desc_off[c + 1])
                        Sc = scp.tile([P, desc[c]], F32, tag="Sc", name=f"SLc{c}")
                        nc.sync.dma_start(out=Sc[:], in_=states[rows, cols])
                        S_chunks.append(Sc)
                    current = chunked_current(S_chunks, spike_t, desc_off)
                else:
                    S = sp.tile([P, in_dim], F32, tag="S")
                    nc.sync.dma_start(out=S[:], in_=states[rows, :])
                    G = gp.tile([P, in_dim], FP8, tag="G")
                    cur = tp.tile([P, 1], F32, tag="cur")
                    nc.vector.scalar_tensor_tensor(
                        out=G[:], in0=S[:], scalar=THRESHOLD, in1=spike_t[:],
                        op0=Op.is_gt, op1=Op.mult, accum_out=cur[:])
                    current = cur

                spk = _small_vec_ops(nc, tp, vec_t, sv_t, current, t, n_tiles)

                # pb[j] = packbits(spikes[8j:8j+8]) via one PE matmul.
                # All compute stays at partition base 0 (hardware requires
                # base 0/32/64); the store DMA places the block at its rows.
                nc.tensor.matmul(pb_ps[:, 0:1], packm_t[:], spk,
                                 start=True, stop=True)
                pb_sb = tp.tile([16, 1], F32, tag="pb_sb")
                nc.vector.tensor_copy(out=pb_sb[:], in_=pb_ps[:, 0:1])
                packT = pkp.tile([16, in_dim], mybir.dt.uint8, tag="packT",
                                 bufs=3)

                if last:
                    nc.sync.dma_start(out=svec[:], in_=sv_t[:])
                    half = in_dim // 2
                    nc.vector.tensor_scalar(out=packT[:, 0:half],
                                            in0=spike_t[0:16, 0:half],
                                            scalar1=pb_sb[:],
                                            scalar2=None, op0=Op.mult)
                    nc.sync.dma_start(out=elig_pack[prows, 0:half],
                                      in_=packT[:, 0:half])
                    nc.scalar.activation(out=packT[:, half:],
                                         in_=spike_t[0:16, half:],
                                         func=Act.Copy, scale=pb_sb[:])
                    nc.sync.dma_start(out=elig_pack[prows, half:],
                                      in_=packT[:, half:])
                else:
                    nc.scalar.activation(out=packT[:, :],
                                         in_=spike_t[0:16, :],
                                         func=Act.Copy, scale=pb_sb[:])
                    nc.scalar.dma_start(out=elig_pack[prows, :],
                                        in_=packT[:, :])

    nc.compile()
    nc._spike_rows = 1
    nc._pack = True
    return nc


def build_general_program(o_shard=OUT_DIM // N_CORES, in_dim=IN_DIM, chunk=4096):
    """Arbitrary inputs, full float32."""
    n_tiles = o_shard // P
    n_chunks = in_dim // chunk
    nv = 3 * n_tiles

    nc = bacc.Bacc("TRN2", target_bir_lowering=False, debug=False)
    states = nc.dram_tensor("states", [o_shard, in_dim], F32, kind="ExternalInput")
    spike_b = nc.dram_tensor("spike_b", [P, in_dim], F32, kind="ExternalInput")
    vecs = nc.dram_tensor("vecs", [P, nv], F32, kind="ExternalInput")
    elig = nc.dram_tensor("elig", [o_shard, in_dim], F32, kind="ExternalInput")
    elig_new = nc.dram_tensor("elig_new", [o_shard, in_dim], F32, kind="ExternalOutput")
    svec = nc.dram_tensor("svec", [P, nv], F32, kind="ExternalOutput")

    with tile.TileContext(nc) as tc:
        with contextlib.ExitStack() as ctx:
            constp = ctx.enter_context(tc.tile_pool(name="constp", bufs=1))
            sp = ctx.enter_context(tc.tile_pool(name="sp", bufs=2))
            gp = ctx.enter_context(tc.tile_pool(name="gp", bufs=1))
            outp = ctx.enter_context(tc.tile_pool(name="outp", bufs=2))
            ep = ctx.enter_context(tc.tile_pool(name="ep", bufs=2))
            e2p = ctx.enter_context(tc.tile_pool(name="e2p", bufs=2))
            tp = ctx.enter_context(tc.tile_pool(name="tp", bufs=3))

            spike_t = constp.tile([P, in_dim], F32, tag="spike_t")
            nc.sync.dma_start(out=spike_t[:], in_=spike_b[:])
            vec_t = constp.tile([P, nv], F32, tag="vec_t")
            nc.sync.dma_start(out=vec_t[:], in_=vecs[:])
            sv_t = constp.tile([P, nv], F32, tag="sv_t")

            for t in range(n_tiles):
                rows = slice(t * P, (t + 1) * P)
                curs = []
                for c in range(n_chunks):
                    cols = slice(c * chunk, (c + 1) * chunk)
                    S = sp.tile([P, chunk], F32, tag="S")
                    nc.sync.dma_start(out=S[:], in_=states[rows, cols])
                    G = gp.tile([P, chunk], F32, tag="G")
                    cur = tp.tile([P, 1], F32, tag="cur", bufs=2 * n_chunks)
                    nc.vector.scalar_tensor_tensor(
                        out=G[:], in0=S[:], scalar=THRESHOLD,
                        in1=spike_t[:, cols],
                        op0=Op.is_gt, op1=Op.mult, accum_out=cur[:])
                    curs.append(cur)
                current = curs[0]
                for other in curs[1:]:
                    acc = tp.tile([P, 1], F32, tag="curacc", bufs=2)
                    nc.vector.tensor_tensor(out=acc[:], in0=current[:],
                                            in1=other[:], op=Op.add)
                    current = acc

                spk = _small_vec_ops(nc, tp, vec_t, sv_t, current, t, n_tiles)

                for c in range(n_chunks):
                    cols = slice(c * chunk, (c + 1) * chunk)
                    O = outp.tile([P, chunk], F32, tag="O")
                    nc.scalar.activation(out=O[:], in_=spike_t[:, cols],
                                         func=Act.Copy, scale=spk)
                    E = ep.tile([P, chunk], F32, tag="E")
                    nc.sync.dma_start(out=E[:], in_=elig[rows, cols])
                    E2 = e2p.tile([P, chunk], F32, tag="E2")
                    nc.vector.scalar_tensor_tensor(
                        out=E2[:], in0=E[:], scalar=0.95, in1=O[:],
                        op0=Op.mult, op1=Op.add)
                    nc.vector.tensor_scalar(out=E2[:], in0=E2[:],
                                            scalar1=0.0, scalar2=5.0,
                                            op0=Op.max, op1=Op.min)
                    nc.scalar.dma_start(out=elig_new[rows, cols], in_=E2[:])

            nc.scalar.dma_start(out=svec[:], in_=sv_t[:])

    nc.compile()
    return nc


_PROGRAM_CACHE = {}
USE_PACK = True


def _get_program(fast: bool):
    key = (fast, USE_PACK)
    if key not in _PROGRAM_CACHE:
        if not fast:
            _PROGRAM_CACHE[key] = build_general_program()
        elif USE_PACK:
            _PROGRAM_CACHE[key] = build_fast_program_pack()
        else:
            _PROGRAM_CACHE[key] = build_fast_program()
    return _PROGRAM_CACHE[key]


def _pack_matrix():
    m = np.zeros((P, 16), dtype=np.float32)
    for j in range(16):
        for k in range(8):
            m[8 * j + k, j] = float(1 << k)
    return m


def _unpack_bits(pk, o_shard, in_dim):
    # pk: [o_shard//8, in_dim] uint8; bit k of row r -> elig row r*8+k
    u = np.unpackbits(pk[:, :, None], axis=2, bitorder="little")
    return u.transpose(0, 2, 1).reshape(o_shard, in_dim)


def _pack_vec(v, n_tiles):
    # [o_shard] -> [128, n_tiles] with column t, partition p = v[t*128+p]
    return np.ascontiguousarray(v.reshape(n_tiles, P).T)


def _unpack_vec(m, n_tiles):
    # inverse of _pack_vec
    return np.ascontiguousarray(m.T).reshape(n_tiles * P)


def run(inputs, trace=False):
    spike_input = np.ascontiguousarray(np.asarray(inputs["spike_input"], dtype=np.float32))
    states = np.ascontiguousarray(np.asarray(inputs["states"], dtype=np.float32))
    v_mem = np.asarray(inputs["v_mem"], dtype=np.float32)
    v_th = np.asarray(inputs["v_th"], dtype=np.float32)
    elig = np.asarray(inputs["elig"], dtype=np.float32)
    noise = np.asarray(inputs["noise"], dtype=np.float32)

    o_shard = OUT_DIM // N_CORES
    n_tiles = o_shard // P

    spike_binary = bool(((spike_input == 0.0) | (spike_input == 1.0)).all())
    fast = (not elig.any()) and spike_binary
    nc = _get_program(fast)

    if fast:
        rows = getattr(nc, "_spike_rows", 1)
        spike_f8 = spike_input.astype(ml_dtypes.float8_e4m3)
        spike_b = np.ascontiguousarray(np.broadcast_to(spike_f8, (rows, IN_DIM)))
    else:
        spike_b = np.ascontiguousarray(np.broadcast_to(spike_input, (P, IN_DIM)))

    in_maps = []
    for c in range(N_CORES):
        rows = slice(c * o_shard, (c + 1) * o_shard)
        vecs = np.concatenate(
            [_pack_vec(v_mem[rows], n_tiles),
             _pack_vec(v_th[rows], n_tiles),
             _pack_vec(noise[rows], n_tiles)], axis=1).astype(np.float32)
        m = {
            "states": np.ascontiguousarray(states[rows]),
            "spike_b": spike_b,
            "vecs": np.ascontiguousarray(vecs),
        }
        if fast and getattr(nc, "_pack", False):
            m["packm"] = _pack_matrix()
        if not fast:
            m["elig"] = np.ascontiguousarray(elig[rows])
        in_maps.append(m)

    res = run_bass_kernel_spmd(nc, in_maps, list(range(N_CORES)), trace=trace)

    spikes = np.empty(OUT_DIM, dtype=np.float32)
    v_mem_new = np.empty(OUT_DIM, dtype=np.float32)
    v_th_new = np.empty(OUT_DIM, dtype=np.float32)
    elig_new = np.empty((OUT_DIM, IN_DIM), dtype=np.float32)
    for c in range(N_CORES):
        rows = slice(c * o_shard, (c + 1) * o_shard)
        out = res.results[c]
        sv = out["svec"]
        spikes[rows] = _unpack_vec(sv[:, 0:n_tiles], n_tiles)
        v_mem_new[rows] = _unpack_vec(sv[:, n_tiles:2 * n_tiles], n_tiles)
        v_th_new[rows] = _unpack_vec(sv[:, 2 * n_tiles:3 * n_tiles], n_tiles)
        if "elig_pack" in out:
            elig_new[rows] = _unpack_bits(np.asarray(out["elig_pack"]),
                                          o_shard, IN_DIM)
        else:
            elig_new[rows] = np.asarray(out["elig_new"]).astype(np.float32)

    return (spikes, v_mem_new, v_th_new, elig_new), res


def kernel(**inputs):
    outputs, _ = run(inputs, trace=False)
    return outputs


# revision 46
# speedup vs baseline: 1.3068x; 1.0276x over previous
"""Trainium2 Bass kernel for nn_LogicGatedSNN.

reference semantics (single step SNN update):
    w        = (states > 50)                      # [O, I] ternary weights
    current  = w @ spike_input                    # [O] GEMV
    v_new    = v_mem * 0.8 + current + noise
    spikes   = (v_new >= v_th)
    elig_new = clip(elig * 0.95 + outer(spikes, spike_input), 0, 5)
    v_th_new = clip(v_th + (spikes - 0.05) * 0.1, 0.5, 10)
    v_mem_new = v_new * (1 - spikes) * 0.2
    returns (spikes, v_mem_new, v_th_new, elig_new)

Sharding: rows (out_features) split across 8 cores; spike_input replicated.
No cross-core communication.

Device kernel per core (o_shard = 1024 rows = 8 tiles of 128 partitions):
    scalar_tensor_tensor: G = (states is_gt 50) * spike_bcast,
                          accum_out = row-sum -> current   (ONE DVE pass)
    tiny [128,1] DVE ops for v_new / spikes / v_th_new / v_mem_new
    outer(spikes, spike) on the Scalar engine:
                          activation(Copy, in=spike_bcast, scale=spikes_col)

Fast mode (dispatched only after the host verifies elig is exactly all-zero
and spike_input is exactly binary): skips the elig read; elig_new values are
then exactly {0,1}, so it is stored as fp8-e4 (exact) and spike_input is
carried as fp8 (exact for binary) and replicated across partitions on the
Tensor engine — pure bandwidth savings, bit-identical results after the host
upcasts to float32. Per-core HBM traffic is 32 MB states in + 8 MB fp8
elig_new out, streamed at ~400 GB/s; typical HW time ~121-130 us.

General mode handles arbitrary inputs in full float32.
"""

import contextlib

import numpy as np
import ml_dtypes

import concourse.bacc as bacc
import concourse.mybir as mybir
from concourse import tile
from concourse.bass_utils import run_bass_kernel_spmd

N_CORES = 8
OUT_DIM = 8192
IN_DIM = 8192
P = 128
THRESHOLD = 50.0

F32 = mybir.dt.float32
BF16 = mybir.dt.bfloat16
FP8 = mybir.dt.float8e4
Op = mybir.AluOpType
Act = mybir.ActivationFunctionType


def _small_vec_ops(nc, tp, vec_t, sv_t, current, t, n_tiles):
    """[128,1] ops for one row-tile: v_new, spikes, v_th_new, v_mem_new.

    Arithmetic matches the reference's op order exactly (bit-identical fp32).
    Returns the spikes column AP.
    """
    vm = vec_t[:, t:t + 1]
    vt = vec_t[:, n_tiles + t:n_tiles + t + 1]
    nz = vec_t[:, 2 * n_tiles + t:2 * n_tiles + t + 1]

    # v_new = (v_mem * 0.8 + current) + noise
    a = tp.tile([P, 1], F32, tag="a")
    nc.vector.scalar_tensor_tensor(out=a[:], in0=vm, scalar=0.8,
                                   in1=current[:], op0=Op.mult, op1=Op.add)
    v_new = tp.tile([P, 1], F32, tag="v_new")
    nc.vector.tensor_tensor(out=v_new[:], in0=a[:], in1=nz, op=Op.add)

    # spikes = v_new >= v_th
    spk = sv_t[:, t:t + 1]
    nc.vector.tensor_tensor(out=spk, in0=v_new[:], in1=vt, op=Op.is_ge)

    # v_th_new = clip(v_th + (spikes - 0.05) * 0.1, 0.5, 10)
    d = tp.tile([P, 1], F32, tag="d")
    nc.vector.tensor_scalar(out=d[:], in0=spk, scalar1=-0.05,
                            scalar2=0.1, op0=Op.add, op1=Op.mult)
    e = tp.tile([P, 1], F32, tag="e")
    nc.vector.tensor_tensor(out=e[:], in0=vt, in1=d[:], op=Op.add)
    nc.vector.tensor_scalar(
        out=sv_t[:, 2 * n_tiles + t:2 * n_tiles + t + 1], in0=e[:],
        scalar1=0.5, scalar2=10.0, op0=Op.max, op1=Op.min)

    # v_mem_new = (v_new * (1 - spikes)) * 0.2
    ns = tp.tile([P, 1], F32, tag="ns")
    nc.vector.tensor_scalar(out=ns[:], in0=spk, scalar1=-1.0,
                            scalar2=1.0, op0=Op.mult, op1=Op.add)
    f = tp.tile([P, 1], F32, tag="f")
    nc.vector.tensor_tensor(out=f[:], in0=v_new[:], in1=ns[:], op=Op.mult)
    nc.vector.tensor_scalar(
        out=sv_t[:, n_tiles + t:n_tiles + t + 1], in0=f[:],
        scalar1=0.2, scalar2=None, op0=Op.mult)
    return spk


def build_fast_program(o_shard=OUT_DIM // N_CORES, in_dim=IN_DIM):
    """elig==0, binary spike_input: skip elig read, fp8 output, fp8 spike.

    Layout: states rows -> 8 tiles of 128 partitions; one fused DVE
    scalar_tensor_tensor per tile computes (states > 50) * spike AND its
    row-sum. The 8 KB spike row is replicated across partitions on the idle
    Tensor engine (rank-1 ones matmul -> PSUM -> ACT copy), saving the 1 MB
    broadcast DMA. The first row-tile is column-chunked (ascending widths)
    so compute starts ~1.3 MB into the load stream; the last row-tile is
    chunked descending and its outer product is split DVE/ACT with stores
    routed to the then-idle sync ring to shorten the post-stream tail.
    elig_new values are exactly {0,1}: stored fp8, host upcasts.
    """
    n_tiles = o_shard // P
    nv = 3 * n_tiles
    asc = [in_dim // 32, in_dim // 8, in_dim * 11 // 32, in_dim // 2]
    NCH = len(asc)
    asc_off = [0]
    for w in asc:
        asc_off.append(asc_off[-1] + w)
    desc = asc[::-1]
    desc_off = [0]
    for w in desc:
        desc_off.append(desc_off[-1] + w)

    nc = bacc.Bacc("TRN2", target_bir_lowering=False, debug=False)
    states = nc.dram_tensor("states", [o_shard, in_dim], F32, kind="ExternalInput")
    spike_b = nc.dram_tensor("spike_b", [1, in_dim], FP8, kind="ExternalInput")
    vecs = nc.dram_tensor("vecs", [P, nv], F32, kind="ExternalInput")
    elig_new = nc.dram_tensor("elig_new", [o_shard, in_dim], FP8, kind="ExternalOutput")
    svec = nc.dram_tensor("svec", [P, nv], F32, kind="ExternalOutput")

    with tile.TileContext(nc) as tc:
        with contextlib.ExitStack() as ctx:
            constp = ctx.enter_context(tc.tile_pool(name="constp", bufs=1))
            sp = ctx.enter_context(tc.tile_pool(name="sp", bufs=3))
            scp = ctx.enter_context(tc.tile_pool(name="scp", bufs=2))
            gp = ctx.enter_context(tc.tile_pool(name="gp", bufs=1))
            outp = ctx.enter_context(tc.tile_pool(name="outp", bufs=2))
            tp = ctx.enter_context(tc.tile_pool(name="tp", bufs=3))

            spike_t = constp.tile([P, in_dim], FP8, tag="spike_t")
            vec_t = constp.tile([P, nv], F32, tag="vec_t")
            sv_t = constp.tile([P, nv], F32, tag="sv_t")

            sp_row = constp.tile([1, in_dim], FP8, tag="sp_row")
            nc.sync.dma_start(out=sp_row[:], in_=spike_b[:])
            ones = constp.tile([1, P], FP8, tag="ones")
            nc.vector.memset(ones[:], 1.0)
            psp = ctx.enter_context(tc.tile_pool(name="psp", bufs=2, space="PSUM"))
            PW = min(2048, in_dim)
            NMM = 512
            for j in range(in_dim // PW):
                pt = psp.tile([P, PW], F32, tag="pt")
                for k in range(PW // NMM):
                    cols = slice(j * PW + k * NMM, j * PW + (k + 1) * NMM)
                    nc.tensor.matmul(pt[:, k * NMM:(k + 1) * NMM],
                                     ones[:], sp_row[:, cols],
                                     start=True, stop=True)
                nc.scalar.activation(out=spike_t[:, j * PW:(j + 1) * PW],
                                     in_=pt[:], func=Act.Copy)

            first_S = [None] * NCH
            first_S[0] = scp.tile([P, asc[0]], F32, tag="Sc", name="S0c0")
            nc.sync.dma_start(out=first_S[0][:], in_=states[0:P, 0:asc[0]])
            nc.sync.dma_start(out=vec_t[:], in_=vecs[:])
            for c in range(1, NCH):
                cols = slice(asc_off[c], asc_off[c + 1])
                first_S[c] = scp.tile([P, asc[c]], F32, tag="Sc", name=f"S0c{c}")
                nc.sync.dma_start(out=first_S[c][:], in_=states[0:P, cols])

            def chunked_current(S_chunks, spike_ap, offs):
                curs = []
                for c, Sc in enumerate(S_chunks):
                    cols = slice(offs[c], offs[c + 1])
                    Gc = gp.tile([P, offs[c + 1] - offs[c]], FP8, tag="Gc",
                                 bufs=2)
                    curc = tp.tile([P, 1], F32, tag="curc", bufs=2 * NCH)
                    nc.vector.scalar_tensor_tensor(
                        out=Gc[:], in0=Sc[:], scalar=THRESHOLD,
                        in1=spike_ap[:, cols],
                        op0=Op.is_gt, op1=Op.mult, accum_out=curc[:])
                    curs.append(curc)
                current = curs[0]
                for other in curs[1:]:
                    acc = tp.tile([P, 1], F32, tag="curacc", bufs=NCH)
                    nc.vector.tensor_tensor(out=acc[:], in0=current[:],
                                            in1=other[:], op=Op.add)
                    current = acc
                return current

            for t in range(n_tiles):
                rows = slice(t * P, (t + 1) * P)
                last = t == n_tiles - 1

                if t == 0:
                    current = chunked_current(first_S, spike_t, asc_off)
                elif last and n_tiles > 1:
                    S_chunks = []
                    for c in range(NCH):
                        cols = slice(desc_off[c], desc_off[c + 1])
                        Sc = scp.tile([P, desc[c]], F32, tag="Sc", name=f"SLc{c}")
                        nc.sync.dma_start(out=Sc[:], in_=states[rows, cols])
                        S_chunks.append(Sc)
                    current = chunked_current(S_chunks, spike_t, desc_off)
                else:
                    S = sp.tile([P, in_dim], F32, tag="S")
                    nc.sync.dma_start(out=S[:], in_=states[rows, :])
                    G = gp.tile([P, in_dim], FP8, tag="G")
                    cur = tp.tile([P, 1], F32, tag="cur")
                    nc.vector.scalar_tensor_tensor(
                        out=G[:], in0=S[:], scalar=THRESHOLD, in1=spike_t[:],
                        op0=Op.is_gt, op1=Op.mult, accum_out=cur[:])
                    current = cur

                spk = _small_vec_ops(nc, tp, vec_t, sv_t, current, t, n_tiles)

                if last:
                    nc.sync.dma_start(out=svec[:], in_=sv_t[:])
                    O = outp.tile([P, in_dim], FP8, tag="O")
                    for c in range(NCH):
                        cols = slice(desc_off[c], desc_off[c + 1])
                        if c < 2:
                            nc.vector.tensor_scalar(out=O[:, cols],
                                                    in0=spike_t[:, cols],
                                                    scalar1=spk, scalar2=None,
                                                    op0=Op.mult)
                        else:
                            nc.scalar.activation(out=O[:, cols],
                                                 in_=spike_t[:, cols],
                                                 func=Act.Copy, scale=spk)
                        nc.sync.dma_start(out=elig_new[rows, cols],
                                          in_=O[:, cols])
                else:
                    O = outp.tile([P, in_dim], FP8, tag="O")
                    nc.scalar.activation(out=O[:], in_=spike_t[:], func=Act.Copy,
                                         scale=spk)
                    nc.scalar.dma_start(out=elig_new[rows, :], in_=O[:])

    nc.compile()
    nc._spike_rows = 1
    nc._w0 = 0
    return nc



def build_fast_program_pack(o_shard=OUT_DIM // N_CORES, in_dim=IN_DIM,
                            scp_bufs=2, dve_t6=False):
    """Like build_fast_program, but elig_new leaves the device bit-packed.

    packbits(outer(spikes, spike)) is itself rank-1: pb[j] * spike[i], where
    pb[j] = sum_k 2^k * spikes[8j+k] — 16 bytes per row-tile, computed by one
    PE matmul (pack-weight matrix @ spikes column, landing on partitions
    16t..16t+15). The ACT outer then emits the packed [16, in_dim] uint8
    block at the same engine cost, cutting the store from 1 MB to 128 KB per
    row-tile. Values are exact integers 0..255; the host unpacks bits.
    """
    n_tiles = o_shard // P
    nv = 3 * n_tiles
    asc = [in_dim // 32, in_dim // 8, in_dim * 11 // 32, in_dim // 2]
    NCH = len(asc)
    asc_off = [0]
    for w in asc:
        asc_off.append(asc_off[-1] + w)
    desc = asc[::-1]
    desc_off = [0]
    for w in desc:
        desc_off.append(desc_off[-1] + w)

    nc = bacc.Bacc("TRN2", target_bir_lowering=False, debug=False)
    states = nc.dram_tensor("states", [o_shard, in_dim], F32, kind="ExternalInput")
    spike_b = nc.dram_tensor("spike_b", [1, in_dim], FP8, kind="ExternalInput")
    vecs = nc.dram_tensor("vecs", [P, nv], F32, kind="ExternalInput")
    packm = nc.dram_tensor("packm", [P, 16], F32, kind="ExternalInput")
    elig_pack = nc.dram_tensor("elig_pack", [n_tiles * 16, in_dim],
                               mybir.dt.uint8, kind="ExternalOutput")
    svec = nc.dram_tensor("svec", [P, nv], F32, kind="ExternalOutput")

    with tile.TileContext(nc) as tc:
        with contextlib.ExitStack() as ctx:
            constp = ctx.enter_context(tc.tile_pool(name="constp", bufs=1))
            sp = ctx.enter_context(tc.tile_pool(name="sp", bufs=3))
            scp = ctx.enter_context(tc.tile_pool(name="scp", bufs=scp_bufs))
            gp = ctx.enter_context(tc.tile_pool(name="gp", bufs=1))
            pkp = ctx.enter_context(tc.tile_pool(name="pkp", bufs=1))
            tp = ctx.enter_context(tc.tile_pool(name="tp", bufs=3))

            spike_t = constp.tile([P, in_dim], FP8, tag="spike_t")
            vec_t = constp.tile([P, nv], F32, tag="vec_t")
            sv_t = constp.tile([P, nv], F32, tag="sv_t")
            packm_t = constp.tile([P, 16], F32, tag="packm_t")

            sp_row = constp.tile([1, in_dim], FP8, tag="sp_row")
            nc.sync.dma_start(out=sp_row[:], in_=spike_b[:])
            nc.sync.dma_start(out=packm_t[:], in_=packm[:])
            ones = constp.tile([1, P], FP8, tag="ones")
            nc.vector.memset(ones[:], 1.0)
            psp = ctx.enter_context(tc.tile_pool(name="psp", bufs=1, space="PSUM"))
            PW = min(2048, in_dim)
            NMM = 512
            for j in range(in_dim // PW):
                pt = psp.tile([P, PW], F32, tag="pt")
                for k in range(PW // NMM):
                    cols = slice(j * PW + k * NMM, j * PW + (k + 1) * NMM)
                    nc.tensor.matmul(pt[:, k * NMM:(k + 1) * NMM],
                                     ones[:], sp_row[:, cols],
                                     start=True, stop=True)
                nc.scalar.activation(out=spike_t[:, j * PW:(j + 1) * PW],
                                     in_=pt[:], func=Act.Copy)
            pb_ps = psp.tile([16, 1], F32, tag="pb_ps", bufs=2)

            first_S = [None] * NCH
            first_S[0] = scp.tile([P, asc[0]], F32, tag="Sc", name="S0c0")
            nc.sync.dma_start(out=first_S[0][:], in_=states[0:P, 0:asc[0]])
            nc.sync.dma_start(out=vec_t[:], in_=vecs[:])
            for c in range(1, NCH):
                cols = slice(asc_off[c], asc_off[c + 1])
                first_S[c] = scp.tile([P, asc[c]], F32, tag="Sc", name=f"S0c{c}")
                nc.sync.dma_start(out=first_S[c][:], in_=states[0:P, cols])

            def chunked_current(S_chunks, spike_ap, offs):
                curs = []
                for c, Sc in enumerate(S_chunks):
                    cols = slice(offs[c], offs[c + 1])
                    Gc = gp.tile([P, offs[c + 1] - offs[c]], FP8, tag="Gc",
                                 bufs=2)
                    curc = tp.tile([P, 1], F32, tag="curc", bufs=2 * NCH)
                    nc.vector.scalar_tensor_tensor(
                        out=Gc[:], in0=Sc[:], scalar=THRESHOLD,
                        in1=spike_ap[:, cols],
                        op0=Op.is_gt, op1=Op.mult, accum_out=curc[:])
                    curs.append(curc)
                current = curs[0]
                for other in curs[1:]:
                    acc = tp.tile([P, 1], F32, tag="curacc", bufs=NCH)
                    nc.vector.tensor_tensor(out=acc[:], in0=current[:],
                                            in1=other[:], op=Op.add)
                    current = acc
                return current

            for t in range(n_tiles):
                rows = slice(t * P, (t + 1) * P)
                last = t == n_tiles - 1
                prows = slice(t * 16, (t + 1) * 16)

                if t == 0:
                    current = chunked_current(first_S, spike_t, asc_off)
                elif last and n_tiles > 1:
                    S_chunks = []
                    for c in range(NCH):
                        cols = slice(desc_off[c], desc_off[c + 1])
                        Sc = scp.tile([P, desc[c]], F32, tag="Sc", name=f"SLc{c}")
                        nc.sync.dma_start(out=Sc[:], in_=states[rows, cols])
                        S_chunks.append(Sc)
                    current = chunked_current(S_chunks, spike_t, desc_off)
                else:
                    S = sp.tile([P, in_dim], F32, tag="S")
                    nc.sync.dma_start(out=S[:], in_=states[rows, :])
                    G = gp.tile([P, in_dim], FP8, tag="G")
                    cur = tp.tile([P, 1], F32, tag="cur")
                    nc.vector.scalar_tensor_tensor(
                        out=G[:], in0=S[:], scalar=THRESHOLD, in1=spike_t[:],
                        op0=Op.is_gt, op1=Op.mult, accum_out=cur[:])
                    current = cur

                spk = _small_vec_ops(nc, tp, vec_t, sv_t, current, t, n_tiles)

                # pb[j] = packbits(spikes[8j:8j+8]) via one PE matmul.
                # All compute stays at partition base 0 (hardware requires
                # base 0/32/64); the store DMA places the block at its rows.
                nc.tensor.matmul(pb_ps[:, 0:1], packm_t[:], spk,
                                 start=True, stop=True)
                pb_sb = tp.tile([16, 1], F32, tag="pb_sb")
                nc.vector.tensor_copy(out=pb_sb[:], in_=pb_ps[:, 0:1])
                packT = pkp.tile([16, in_dim], mybir.dt.uint8, tag="packT",
                                 bufs=3)

                if last:
                    nc.sync.dma_start(out=svec[:], in_=sv_t[:])
                    half = in_dim // 2
                    nc.vector.tensor_scalar(out=packT[:, 0:half],
                                            in0=spike_t[0:16, 0:half],
                                            scalar1=pb_sb[:],
                                            scalar2=None, op0=Op.mult)
                    nc.sync.dma_start(out=elig_pack[prows, 0:half],
                                      in_=packT[:, 0:half])
                    nc.vector.tensor_scalar(out=packT[:, half:],
                                            in0=spike_t[0:16, half:],
                                            scalar1=pb_sb[:],
                                            scalar2=None, op0=Op.mult)
                    nc.sync.dma_start(out=elig_pack[prows, half:],
                                      in_=packT[:, half:])
                elif dve_t6 and t == n_tiles - 2 and n_tiles > 2:
                    # DVE is idle here and ACT would otherwise still be busy
                    # with this outer when the last tile's tail needs it
                    nc.vector.tensor_scalar(out=packT[:, :],
                                            in0=spike_t[0:16, :],
                                            scalar1=pb_sb[:],
                                            scalar2=None, op0=Op.mult)
                    nc.scalar.dma_start(out=elig_pack[prows, :],
                                        in_=packT[:, :])
                else:
                    nc.scalar.activation(out=packT[:, :],
                                         in_=spike_t[0:16, :],
                                         func=Act.Copy, scale=pb_sb[:])
                    nc.scalar.dma_start(out=elig_pack[prows, :],
                                        in_=packT[:, :])

    nc.compile()
    nc._spike_rows = 1
    nc._pack = True
    return nc


def build_general_program(o_shard=OUT_DIM // N_CORES, in_dim=IN_DIM, chunk=4096):
    """Arbitrary inputs, full float32."""
    n_tiles = o_shard // P
    n_chunks = in_dim // chunk
    nv = 3 * n_tiles

    nc = bacc.Bacc("TRN2", target_bir_lowering=False, debug=False)
    states = nc.dram_tensor("states", [o_shard, in_dim], F32, kind="ExternalInput")
    spike_b = nc.dram_tensor("spike_b", [P, in_dim], F32, kind="ExternalInput")
    vecs = nc.dram_tensor("vecs", [P, nv], F32, kind="ExternalInput")
    elig = nc.dram_tensor("elig", [o_shard, in_dim], F32, kind="ExternalInput")
    elig_new = nc.dram_tensor("elig_new", [o_shard, in_dim], F32, kind="ExternalOutput")
    svec = nc.dram_tensor("svec", [P, nv], F32, kind="ExternalOutput")

    with tile.TileContext(nc) as tc:
        with contextlib.ExitStack() as ctx:
            constp = ctx.enter_context(tc.tile_pool(name="constp", bufs=1))
            sp = ctx.enter_context(tc.tile_pool(name="sp", bufs=2))
            gp = ctx.enter_context(tc.tile_pool(name="gp", bufs=1))
            outp = ctx.enter_context(tc.tile_pool(name="outp", bufs=2))
            ep = ctx.enter_context(tc.tile_pool(name="ep", bufs=2))
            e2p = ctx.enter_context(tc.tile_pool(name="e2p", bufs=2))
            tp = ctx.enter_context(tc.tile_pool(name="tp", bufs=3))

            spike_t = constp.tile([P, in_dim], F32, tag="spike_t")
            nc.sync.dma_start(out=spike_t[:], in_=spike_b[:])
            vec_t = constp.tile([P, nv], F32, tag="vec_t")
            nc.sync.dma_start(out=vec_t[:], in_=vecs[:])
            sv_t = constp.tile([P, nv], F32, tag="sv_t")

            for t in range(n_tiles):
                rows = slice(t * P, (t + 1) * P)
                curs = []
                for c in range(n_chunks):
                    cols = slice(c * chunk, (c + 1) * chunk)
                    S = sp.tile([P, chunk], F32, tag="S")
                    nc.sync.dma_start(out=S[:], in_=states[rows, cols])
                    G = gp.tile([P, chunk], F32, tag="G")
                    cur = tp.tile([P, 1], F32, tag="cur", bufs=2 * n_chunks)
                    nc.vector.scalar_tensor_tensor(
                        out=G[:], in0=S[:], scalar=THRESHOLD,
                        in1=spike_t[:, cols],
                        op0=Op.is_gt, op1=Op.mult, accum_out=cur[:])
                    curs.append(cur)
                current = curs[0]
                for other in curs[1:]:
                    acc = tp.tile([P, 1], F32, tag="curacc", bufs=2)
                    nc.vector.tensor_tensor(out=acc[:], in0=current[:],
                                            in1=other[:], op=Op.add)
                    current = acc

                spk = _small_vec_ops(nc, tp, vec_t, sv_t, current, t, n_tiles)

                for c in range(n_chunks):
                    cols = slice(c * chunk, (c + 1) * chunk)
                    O = outp.tile([P, chunk], F32, tag="O")
                    nc.scalar.activation(out=O[:], in_=spike_t[:, cols],
                                         func=Act.Copy, scale=spk)
                    E = ep.tile([P, chunk], F32, tag="E")
                    nc.sync.dma_start(out=E[:], in_=elig[rows, cols])
                    E2 = e2p.tile([P, chunk], F32, tag="E2")
                    nc.vector.scalar_tensor_tensor(
                        out=E2[:], in0=E[:], scalar=0.95, in1=O[:],
                        op0=Op.mult, op1=Op.add)
                    nc.vector.tensor_scalar(out=E2[:], in0=E2[:],
                                            scalar1=0.0, scalar2=5.0,
                                            op0=Op.max, op1=Op.min)
                    nc.scalar.dma_start(out=elig_new[rows, cols], in_=E2[:])

            nc.scalar.dma_start(out=svec[:], in_=sv_t[:])

    nc.compile()
    return nc


_PROGRAM_CACHE = {}
USE_PACK = True


def _get_program(fast: bool):
    key = (fast, USE_PACK)
    if key not in _PROGRAM_CACHE:
        if not fast:
            _PROGRAM_CACHE[key] = build_general_program()
        elif USE_PACK:
            _PROGRAM_CACHE[key] = build_fast_program_pack()
        else:
            _PROGRAM_CACHE[key] = build_fast_program()
    return _PROGRAM_CACHE[key]


def _pack_matrix():
    m = np.zeros((P, 16), dtype=np.float32)
    for j in range(16):
        for k in range(8):
            m[8 * j + k, j] = float(1 << k)
    return m


def _unpack_bits(pk, o_shard, in_dim):
    # pk: [o_shard//8, in_dim] uint8; bit k of row r -> elig row r*8+k
    u = np.unpackbits(pk[:, :, None], axis=2, bitorder="little")
    return u.transpose(0, 2, 1).reshape(o_shard, in_dim)


def _pack_vec(v, n_tiles):
    # [o_shard] -> [128, n_tiles] with column t, partition p = v[t*128+p]
    return np.ascontiguousarray(v.reshape(n_tiles, P).T)


def _unpack_vec(m, n_tiles):
    # inverse of _pack_vec
    return np.ascontiguousarray(m.T).reshape(n_tiles * P)


def run(inputs, trace=False):
    spike_input = np.ascontiguousarray(np.asarray(inputs["spike_input"], dtype=np.float32))
    states = np.ascontiguousarray(np.asarray(inputs["states"], dtype=np.float32))
    v_mem = np.asarray(inputs["v_mem"], dtype=np.float32)
    v_th = np.asarray(inputs["v_th"], dtype=np.float32)
    elig = np.asarray(inputs["elig"], dtype=np.float32)
    noise = np.asarray(inputs["noise"], dtype=np.float32)

    o_shard = OUT_DIM // N_CORES
    n_tiles = o_shard // P

    spike_binary = bool(((spike_input == 0.0) | (spike_input == 1.0)).all())
    fast = (not elig.any()) and spike_binary
    nc = _get_program(fast)

    if fast:
        rows = getattr(nc, "_spike_rows", 1)
        spike_f8 = spike_input.astype(ml_dtypes.float8_e4m3)
        spike_b = np.ascontiguousarray(np.broadcast_to(spike_f8, (rows, IN_DIM)))
    else:
        spike_b = np.ascontiguousarray(np.broadcast_to(spike_input, (P, IN_DIM)))

    in_maps = []
    for c in range(N_CORES):
        rows = slice(c * o_shard, (c + 1) * o_shard)
        vecs = np.concatenate(
            [_pack_vec(v_mem[rows], n_tiles),
             _pack_vec(v_th[rows], n_tiles),
             _pack_vec(noise[rows], n_tiles)], axis=1).astype(np.float32)
        m = {
            "states": np.ascontiguousarray(states[rows]),
            "spike_b": spike_b,
            "vecs": np.ascontiguousarray(vecs),
        }
        if fast and getattr(nc, "_pack", False):
            m["packm"] = _pack_matrix()
        if not fast:
            m["elig"] = np.ascontiguousarray(elig[rows])
        in_maps.append(m)

    res = run_bass_kernel_spmd(nc, in_maps, list(range(N_CORES)), trace=trace)

    spikes = np.empty(OUT_DIM, dtype=np.float32)
    v_mem_new = np.empty(OUT_DIM, dtype=np.float32)
    v_th_new = np.empty(OUT_DIM, dtype=np.float32)
    elig_new = np.empty((OUT_DIM, IN_DIM), dtype=np.float32)
    for c in range(N_CORES):
        rows = slice(c * o_shard, (c + 1) * o_shard)
        out = res.results[c]
        sv = out["svec"]
        spikes[rows] = _unpack_vec(sv[:, 0:n_tiles], n_tiles)
        v_mem_new[rows] = _unpack_vec(sv[:, n_tiles:2 * n_tiles], n_tiles)
        v_th_new[rows] = _unpack_vec(sv[:, 2 * n_tiles:3 * n_tiles], n_tiles)
        if "elig_pack" in out:
            elig_new[rows] = _unpack_bits(np.asarray(out["elig_pack"]),
                                          o_shard, IN_DIM)
        else:
            elig_new[rows] = np.asarray(out["elig_new"]).astype(np.float32)

    return (spikes, v_mem_new, v_th_new, elig_new), res


def kernel(**inputs):
    outputs, _ = run(inputs, trace=False)
    return outputs
